# revision 33
# baseline (speedup 1.0000x reference)
"""Trainium2 Bass kernel for a 4-layer dense transformer (B=1, S=2048, D=1024,
H=16, DK=64, FF=4096, V=50000) distributed over 8 NeuronCores.

Sharding:
  - Attention: tensor-parallel over heads (2 heads/core), full sequence.
  - LayerNorm / FFN / residual: sequence-parallel (256 rows/core), full width.
  - Vocab projection: sharded over vocab (6250 cols/core).
  - Glue per layer: AllGather of x^T (for QKV inputs) and AllToAll of the
    normalized ctx^T (delivers every head's dims for the core's own rows).
    One final AllGather before the vocab matmul.

v2: all matmul operands in bf16 (FWL weight loads, half DMA/collective
traffic), host-side weight re-tiling for contiguous >=2KB DMA descriptors,
two-head-batched softmax exp, LN rstd via ln+exp (stays in the exp ACT
table set), RoPE in bf16 with DMA-based rotate-half shuffle and sign-folded
sin tables. Residual/LN/softmax accumulation stays fp32.
"""
import sys

if "/opt/trn_rl_repo" not in sys.path:
    sys.path.insert(0, "/opt/trn_rl_repo")

import contextlib

import ml_dtypes
import numpy as np

import concourse.bass as bass
import concourse.tile as tile
from concourse import bacc, mybir
from concourse.bass_utils import run_bass_kernel_spmd
from concourse.masks import make_identity

F32 = mybir.dt.float32
BF16 = mybir.dt.bfloat16
I32 = mybir.dt.int32
AF = mybir.ActivationFunctionType
BF_NP = ml_dtypes.bfloat16

NC = 8                    # cores
B, S, D, H, DK, FF, V, L = 1, 2048, 1024, 16, 64, 4096, 50000, 4
EPS = 1e-5
SCALE = 1.0 / np.sqrt(DK)
HL = H // NC              # heads per core = 2
DHL = HL * DK             # local head dims = 128
SL = S // NC              # rows per core = 256
VL = V // NC              # vocab per core = 6250
KC = D // 128             # contraction chunks over D = 8
NVC = 13                  # vocab chunks of 512 (last is 106 real)
VCL = 106                 # real cols in last chunk

_CACHE = {}


def _np_rope_tables():
    inv_freq = 1.0 / (10000.0 ** (np.arange(0, DK, 2, dtype=np.float32) / DK))
    t = np.arange(S, dtype=np.float32)
    freqs = np.outer(t, inv_freq)                 # [S, DK/2]
    emb = np.concatenate([freqs, freqs], -1)      # [S, DK]
    return np.cos(emb), np.sin(emb)


def _diag_masks():
    # Triangle mask for the 128x128 block straddling the diagonal: within
    # cols [128d, 128d+128) of a diag chunk, allowed iff local col j >= k.
    # Identical for every d; stored per-head-duplicated: [128, 4, 2, 128].
    masks = np.zeros((128, 4, 2, 128), np.float32)
    k = np.arange(128)[:, None]
    j = np.arange(128)[None, :]
    m = (j >= k).astype(np.float32)
    for d in range(4):
        masks[:, d, 0, :] = m
        masks[:, d, 1, :] = m
    return masks


def build_program(n_layers=L, debug_x=False):
    nc = bacc.Bacc("TRN2", target_bir_lowering=False, debug=False,
                   num_devices=NC)

    t = {}
    t["ids"] = nc.dram_tensor("ids", [2, 128, 1], I32, kind="ExternalInput")
    t["temb"] = nc.dram_tensor("token_emb", [V, D], BF16, kind="ExternalInput")
    t["pemb"] = nc.dram_tensor("pos_emb", [SL, D], F32, kind="ExternalInput")
    # weights pre-tiled on host: per-partition contiguous
    t["wq"] = nc.dram_tensor("wq", [L, 128, KC, DHL], BF16,
                             kind="ExternalInput")
    t["wk"] = nc.dram_tensor("wk", [L, 128, KC, DHL], BF16,
                             kind="ExternalInput")
    t["wv"] = nc.dram_tensor("wv", [L, 128, KC, DHL], BF16,
                             kind="ExternalInput")
    t["wo"] = nc.dram_tensor("wo_w", [L, 128, KC, D], BF16,
                             kind="ExternalInput")
    t["wob"] = nc.dram_tensor("wo_b", [L, D], F32, kind="ExternalInput")
    t["ln1w"] = nc.dram_tensor("ln1_w", [L, D], F32, kind="ExternalInput")
    t["ln1b"] = nc.dram_tensor("ln1_b", [L, D], F32, kind="ExternalInput")
    t["ln2w"] = nc.dram_tensor("ln2_w", [L, D], F32, kind="ExternalInput")
    t["ln2b"] = nc.dram_tensor("ln2_b", [L, D], F32, kind="ExternalInput")
    t["ff1"] = nc.dram_tensor("ff1_w", [L, FF // 128, 128, KC, 128], BF16,
                              kind="ExternalInput")
    t["ff1b"] = nc.dram_tensor("ff1_b", [L, 128, FF // 128], F32,
                               kind="ExternalInput")
    t["ff2"] = nc.dram_tensor("ff2_w", [L, 4, 128, KC, D], BF16,
                              kind="ExternalInput")
    t["ff2b"] = nc.dram_tensor("ff2_b", [L, D], F32, kind="ExternalInput")
    t["outw"] = nc.dram_tensor("out_w", [NVC, 128, KC, 512], BF16,
                               kind="ExternalInput")
    t["outb"] = nc.dram_tensor("out_b", [NVC, 512], F32, kind="ExternalInput")
    t["cos"] = nc.dram_tensor("cosT", [128, S], BF16, kind="ExternalInput")
    t["sinm"] = nc.dram_tensor("sinmT", [128, S], BF16, kind="ExternalInput")
    t["dmask"] = nc.dram_tensor("dmask", [128, 4, 2, 128], BF16,
                                kind="ExternalInput")

    t["logits"] = nc.dram_tensor("logits", [S, VL], F32, kind="ExternalOutput")
    if debug_x:
        t["dbg_x"] = nc.dram_tensor("dbg_x", [SL, D], F32,
                                    kind="ExternalOutput")

    # collective bounce buffers (bf16), split for compute/collective overlap
    t["comm_in"] = nc.dram_tensor("comm_in", [16], BF16)
    t["comm_out"] = nc.dram_tensor("comm_out", [NC, 16], BF16,
                                   addr_space="Shared")
    t["xt_in_b"] = [nc.dram_tensor(f"xt_in_{l}", [128, KC, SL], BF16)
                    for l in range(n_layers + 1)]
    t["xt_out_b"] = [nc.dram_tensor(f"xt_out_{l}", [NC, 128, KC, SL], BF16,
                                    addr_space="Shared")
                     for l in range(n_layers + 1)]
    t["cx_in_b"] = [nc.dram_tensor(f"cx_in_{l}", [NC, 128, SL], BF16)
                    for l in range(n_layers)]
    t["cx_out_b"] = [nc.dram_tensor(f"cx_out_{l}", [NC, 128, SL], BF16)
                     for l in range(n_layers)]

    with tile.TileContext(nc) as tc:
        _build(nc, tc, t, n_layers, debug_x)
    nc.compile()
    return nc


def _build(nc, tc, t, n_layers, debug_x):
    rg = [list(range(NC))]
    es = contextlib.ExitStack()
    with es:
        const = es.enter_context(tc.tile_pool(name="const", bufs=1))
        glob = es.enter_context(tc.tile_pool(name="glob", bufs=1))

        # ---------------- constants ----------------
        ident = const.tile([128, 128], F32)
        make_identity(nc, ident[:])
        cos_t = const.tile([128, S], BF16)
        sinm_t = const.tile([128, S], BF16)
        nc.sync.dma_start(cos_t[:], t["cos"][:, :])
        nc.sync.dma_start(sinm_t[:], t["sinm"][:, :])
        dmask_t = const.tile([128, 4, 2, 128], BF16)
        nc.sync.dma_start(dmask_t[:], t["dmask"][:, :, :, :])
        ones_t = const.tile([128, 16], BF16)
        nc.vector.memset(ones_t[:], 1.0)
        eps_t = const.tile([128, 1], F32)
        nc.vector.memset(eps_t[:], EPS)
        warm_src = const.tile([128, 512], BF16)
        nc.vector.memset(warm_src[:], 0.001)

        def warm_chain(psum_pool, dep_ap, n):
            """Keep the PE busy with dummy matmuls while a collective runs.

            dep_ap gates the chain start (SBUF tile written just before the
            gap); WAW on the psum tile serializes the chain so it spans the
            idle window instead of racing ahead. Keeps the HAM activity
            monitor from dropping the PE clock to 1.2GHz."""
            wp = psum_pool.tile([128, 512], F32, name="warm")
            for _ in range(n):
                nc.tensor.matmul(wp[:], dep_ap, warm_src[:],
                                 start=True, stop=True)

        def bcast_load(dst, src_1d):
            """DMA a [N] DRAM vector into a [P, N] tile, replicated."""
            p = dst.shape[0]
            ap = bass.AP(tensor=src_1d.tensor, offset=src_1d.offset,
                         ap=[[0, p]] + src_1d.ap)
            nc.sync.dma_start(dst, ap)

        # x_own[m]: [128, 1024] f32, own rows (m=0: rows 0..127 of the
        # core's 256; m=1: rows 128..255)
        x_own = [glob.tile([128, D], F32, name=f"x_own{m}") for m in range(2)]

        def gather_xt(src_tiles, lx, pool, tp_pool, warm_pool=None,
                      warm_n=0):
            """src [2][128, 1024] f32 -> xt bounce (bf16, via PE
            transpose) followed by its AllGather."""
            xt_sb = pool.tile([128, KC, 256], BF16, name="xt_sb")
            for kc in range(KC):
                for m in range(2):
                    tp = tp_pool.tile([128, 128], F32, name="tp_ps")
                    nc.tensor.transpose(
                        tp[:], src_tiles[m][:, kc * 128:(kc + 1) * 128],
                        ident[:])
                    nc.vector.tensor_copy(
                        xt_sb[:, kc, m * 128:(m + 1) * 128], tp[:])
            nc.sync.dma_start(t["xt_in_b"][lx][:, :, :], xt_sb[:])
            nc.gpsimd.collective_compute(
                "AllGather", mybir.AluOpType.bypass, replica_groups=rg,
                ins=[t["xt_in_b"][lx][:, :, :]],
                outs=[t["xt_out_b"][lx][:, :, :, :]])
            if warm_n:
                warm_chain(warm_pool, xt_sb[:, 0, 0:128], warm_n)

        def layer_norm(dst, src, w_t, b_t, small, eng=None):
            eng = eng or nc.vector
            st = small.tile([128, 2, 6], F32, name="bn_st")
            mv = small.tile([128, 2], F32, name="bn_mv")
            for g in range(2):
                nc.vector.bn_stats(st[:, g, :],
                                   src[:, g * 512:(g + 1) * 512])
            nc.vector.bn_aggr(mv[:], st[:])
            rstd = small.tile([128, 1], F32, name="rstd")
            nc.scalar.activation(rstd[:], mv[:, 1:2], AF.Sqrt, bias=eps_t[:])
            nc.vector.reciprocal(rstd[:], rstd[:])
            eng.tensor_scalar(
                out=dst[:], in0=src[:], scalar1=mv[:, 0:1], scalar2=rstd[:],
                op0=mybir.AluOpType.subtract, op1=mybir.AluOpType.mult)
            eng.tensor_mul(dst[:], dst[:], w_t[:])
            eng.tensor_add(dst[:], dst[:], b_t[:])

        # ---------------- embedding ----------------
        with nc.named_scope("embed"):
            # fire a tiny collective first: the ~100us cross-core comm-init
            # barrier runs while the embedding computes
            nc.vector.memset(ones_t[:, 0:1], 1.0)
            nc.sync.dma_start(t["comm_in"][:], ones_t[0:1, 0:16])
            nc.gpsimd.collective_compute(
                "AllGather", mybir.AluOpType.bypass, replica_groups=rg,
                ins=[t["comm_in"][:]], outs=[t["comm_out"][:, :]])
            with tc.tile_pool(name="emb", bufs=2) as emb:
                for m in range(2):
                    idx_t = emb.tile([128, 1], I32, name="idx")
                    nc.sync.dma_start(idx_t[:], t["ids"][m, :, :])
                    gat = emb.tile([128, D], BF16, name="gat")
                    nc.gpsimd.indirect_dma_start(
                        out=gat[:], out_offset=None, in_=t["temb"][:, :],
                        in_offset=bass.IndirectOffsetOnAxis(ap=idx_t[:, :1],
                                                            axis=0))
                    pos_t = emb.tile([128, D], F32, name="pos")
                    nc.sync.dma_start(pos_t[:],
                                      t["pemb"][m * 128:(m + 1) * 128, :])
                    nc.vector.tensor_add(x_own[m][:], gat[:], pos_t[:])
                with tc.tile_pool(name="ps_we", bufs=1, space="PSUM") as pwm, \
                     tc.tile_pool(name="ps_te", bufs=2, space="PSUM") as ptp:
                    gather_xt(x_own, 0, emb, ptp, pwm, 240)

        # ---------------- layers ----------------
        for l in range(n_layers):
            with tc.tile_pool(name=f"layer{l}", bufs=1) as lp:
                qTr = lp.tile([128, S], BF16, name="qTr")
                kTr = lp.tile([128, S], BF16, name="kTr")
                v_aug = [lp.tile([128, 16, 65], BF16, name=f"vaug{h}")
                         for h in range(HL)]
                ctxc = lp.tile([128, S], BF16, name="ctxc")
                xln = [lp.tile([128, D], F32, name=f"xln{m}")
                       for m in range(2)]
                z = [lp.tile([128, D], F32, name=f"zz{m}") for m in range(2)]
                # prefetch out-proj weights/params at layer start
                wo_sb = lp.tile([128, KC, D], BF16, name="wo_sb")
                nc.sync.dma_start(wo_sb[:], t["wo"][l])
                wob_t = lp.tile([128, D], F32, name="wob")
                bcast_load(wob_t[:], t["wob"][l])
                ln1w_t = lp.tile([128, D], F32, name="ln1w")
                ln1b_t = lp.tile([128, D], F32, name="ln1b")
                bcast_load(ln1w_t[:], t["ln1w"][l])
                bcast_load(ln1b_t[:], t["ln1b"][l])

                # ---- phase 1: QKV (+ inline RoPE per 512-col chunk) ----
                with nc.named_scope(f"L{l}.qkv"), \
                     tc.tile_pool(name="ph_qkv", bufs=2) as pp, \
                     tc.tile_pool(name="qkv_qk", bufs=1) as qk, \
                     tc.tile_pool(name="ps_qkv", bufs=3, space="PSUM") as pq:
                    wq_sb = pp.tile([128, KC, DHL], BF16, name="wq_sb")
                    wk_sb = pp.tile([128, KC, DHL], BF16, name="wk_sb")
                    wv_sb = pp.tile([128, KC, DHL], BF16, name="wv_sb")
                    nc.sync.dma_start(wq_sb[:], t["wq"][l])
                    nc.sync.dma_start(wk_sb[:], t["wk"][l])
                    nc.sync.dma_start(wv_sb[:], t["wv"][l])
                    for h in range(HL):
                        nc.vector.tensor_copy(v_aug[h][:, :, 64], ones_t[:])
                    qT = qk.tile([128, S], BF16, name="qT")
                    kT = qk.tile([128, S], BF16, name="kT")
                    for rb in range(NC):
                        ssl = slice(rb * 256, (rb + 1) * 256)
                        xt_c = pp.tile([128, KC, 256], BF16, name="xt_c")
                        nc.gpsimd.dma_start(xt_c[:], t["xt_out_b"][l][rb])
                        for ci, (w_sb, dstT) in enumerate(
                                ((wk_sb, kT), (wq_sb, qT))):
                            pt = pq.tile([128, 256], F32, name="qk_ps")
                            for kc in range(KC):
                                nc.tensor.matmul(
                                    pt[:], w_sb[:, kc, :],
                                    xt_c[:, kc, :],
                                    start=(kc == 0), stop=(kc == KC - 1))
                            if ci == 0:
                                nc.scalar.copy(dstT[:, ssl], pt[:])
                            else:
                                nc.vector.tensor_copy(dstT[:, ssl], pt[:])
                        for half in range(2):
                            sc = rb * 2 + half
                            pt = pq.tile([128, DHL], F32, name="v_ps")
                            for kc in range(KC):
                                nc.tensor.matmul(
                                    pt[:],
                                    xt_c[:, kc, half * 128:(half + 1) * 128],
                                    wv_sb[:, kc, :],
                                    start=(kc == 0), stop=(kc == KC - 1))
                            for h in range(HL):
                                nc.scalar.copy(
                                    v_aug[h][:, sc, 0:64],
                                    pt[:, h * 64:(h + 1) * 64])
                        # RoPE on the completed 512-col chunk (rb odd)
                        if rb % 2 == 1:
                            cb = rb // 2
                            csl = slice(cb * 512, (cb + 1) * 512)
                            for srcT, dstT in ((kT, kTr), (qT, qTr)):
                                rh = pp.tile([128, 512], BF16, name="rope_rh")
                                for h in range(HL):
                                    lo = h * 64
                                    hi = h * 64 + 32
                                    nc.sync.dma_start(rh[lo:hi, 0:512],
                                                      srcT[hi:hi + 32, csl])
                                    nc.sync.dma_start(rh[hi:hi + 32, 0:512],
                                                      srcT[lo:hi, csl])
                                nc.gpsimd.tensor_mul(
                                    rh[:], rh[:], sinm_t[:, csl])
                                tmp = pp.tile([128, 512], BF16, name="rope_t2")
                                nc.vector.tensor_mul(
                                    tmp[:], srcT[:, csl], cos_t[:, csl])
                                nc.vector.tensor_add(dstT[:, csl], tmp[:],
                                                     rh[:])

                # ---- phase 2: attention ----
                with nc.named_scope(f"L{l}.att"), \
                     tc.tile_pool(name="ph_att", bufs=3) as pp, \
                     tc.tile_pool(name="ps_sc", bufs=2, space="PSUM") as psc, \
                     tc.tile_pool(name="ps_ct", bufs=2, space="PSUM") as pct, \
                     tc.tile_pool(name="att_sm", bufs=3) as sm:
                    for qb in range(4):
                        nkc = (qb + 1) * 4
                        qsl = slice(qb * 512, (qb + 1) * 512)
                        ct_ps = [pct.tile([65, 512], F32, name=f"ct_ps{h}")
                                 for h in range(HL)]
                        for kc in range(nkc):
                            ksl = slice(kc * 128, (kc + 1) * 128)
                            sc_ps = psc.tile([128, 2, 512], F32, name="sc_ps")
                            for h in range(HL):
                                nc.tensor.matmul(
                                    sc_ps[:, h, :],
                                    kTr[h * 64:(h + 1) * 64, ksl],
                                    qTr[h * 64:(h + 1) * 64, qsl],
                                    start=True, stop=True,
                                    tile_position=(64 * h, 0))
                            et = pp.tile([128, 2, 512], BF16, name="exp")
                            d = kc - qb * 4
                            # diag chunk d: cols < 128*d are fully masked —
                            # skip their exp (the mask mult zeroes stale
                            # data; every et slot sees a full-range write
                            # first, so stale values are finite).
                            c0 = 128 * d if d > 0 else 0
                            nc.scalar.activation(et[:, :, c0:],
                                                 sc_ps[:, :, c0:],
                                                 AF.Exp, scale=float(SCALE))
                            if d > 0:
                                nc.gpsimd.memset(et[:, :, 0:c0], 0.0)
                            if d >= 0:
                                nc.vector.tensor_mul(
                                    et[:, :, c0:c0 + 128], et[:, :, c0:c0 + 128],
                                    dmask_t[:, d, :, :])
                            for h in range(HL):
                                nc.tensor.matmul(
                                    ct_ps[h][:], v_aug[h][:, kc, :],
                                    et[:, h, :],
                                    start=(kc == 0), stop=(kc == nkc - 1))
                        for h in range(HL):
                            den = sm.tile([1, 512], F32, name="den")
                            nc.vector.tensor_copy(den[:], ct_ps[h][64:65, :])
                            rec = sm.tile([1, 512], F32, name="rec")
                            nc.vector.reciprocal_approx_fast(rec[:], den[:])
                            bc = sm.tile([64, 512], F32, name="bc")
                            nc.gpsimd.partition_broadcast(bc[:], rec[:])
                            nc.vector.tensor_mul(
                                ctxc[h * 64:(h + 1) * 64, qsl],
                                ct_ps[h][0:64, :], bc[:])

                # ---- AllToAll ctx: shard j = ctxc[:, j*256:(j+1)*256] ----
                with nc.named_scope(f"L{l}.a2a"):
                    for j in range(NC):
                        nc.sync.dma_start(t["cx_in_b"][l][j],
                                          ctxc[:, j * SL:(j + 1) * SL])
                    nc.gpsimd.collective_compute(
                        "AllToAll", mybir.AluOpType.bypass, replica_groups=rg,
                        ins=[t["cx_in_b"][l][:, :, :]],
                        outs=[t["cx_out_b"][l][:, :, :]])

                # ---- phase 3: out-proj + LN1 ----
                with nc.named_scope(f"L{l}.wo"), \
                     tc.tile_pool(name="ph_wo", bufs=1) as pp, \
                     tc.tile_pool(name="wo_sm", bufs=3) as sm, \
                     tc.tile_pool(name="ps_wo", bufs=2, space="PSUM") as pw:
                    warm_chain(pw, ctxc[:, S - 128:S], 90)
                    ctxT = pp.tile([128, KC, 256], BF16, name="ctxT")
                    for rb in range(NC):
                        nc.gpsimd.dma_start(ctxT[:, rb, :],
                                            t["cx_out_b"][l][rb])
                    for m in range(2):
                        for n in range(2):
                            yp = pw.tile([128, 512], F32, name="y_ps")
                            for kc in range(KC):
                                nc.tensor.matmul(
                                    yp[:], ctxT[:, kc, m * 128:(m + 1) * 128],
                                    wo_sb[:, kc, n * 512:(n + 1) * 512],
                                    start=(kc == 0), stop=(kc == KC - 1))
                            nsl = slice(n * 512, (n + 1) * 512)
                            nc.vector.tensor_add(z[m][:, nsl], yp[:],
                                                 x_own[m][:, nsl])
                        nc.vector.tensor_add(z[m][:], z[m][:], wob_t[:])
                        if m == 0:
                            g1 = sm.tile([128, 128], BF16, name="wgate1")
                            nc.vector.tensor_copy(g1[:], z[0][:, 0:128])
                            warm_chain(pw, g1[:], 55)
                        layer_norm(xln[m], z[m], ln1w_t, ln1b_t, sm,
                                   nc.gpsimd if m == 1 else nc.vector)

                # ---- phase 4: FFN + LN2 ----
                with nc.named_scope(f"L{l}.ffn"), \
                     tc.tile_pool(name="ph_ff", bufs=1) as pp, \
                     tc.tile_pool(name="ff_st", bufs=3) as st, \
                     tc.tile_pool(name="ff_sm", bufs=3) as sm, \
                     tc.tile_pool(name="ps_h", bufs=2, space="PSUM") as ph, \
                     tc.tile_pool(name="ps_tf", bufs=2, space="PSUM") as ptp, \
                     tc.tile_pool(name="ps_y2", bufs=1, space="PSUM") as py2:
                    xlnT = pp.tile([128, KC, 256], BF16, name="xlnT")
                    for m in range(2):
                        for kc in range(KC):
                            tp = ptp.tile([128, 128], F32, name="tp_ps")
                            nc.tensor.transpose(
                                tp[:], xln[m][:, kc * 128:(kc + 1) * 128],
                                ident[:])
                            nc.vector.tensor_copy(
                                xlnT[:, kc, m * 128:(m + 1) * 128], tp[:])
                    ff1b_t = sm.tile([128, FF // 128], F32, name="ff1b")
                    nc.sync.dma_start(ff1b_t[:], t["ff1b"][l])
                    hT = pp.tile([128, FF // 128, 256], BF16, name="hT")
                    for mh in range(FF // 128):
                        f1t = st.tile([128, KC, 128], BF16, name="f1t")
                        nc.sync.dma_start(f1t[:], t["ff1"][l, mh])
                        hp = ph.tile([128, 256], F32, name="h_ps")
                        for kc in range(KC):
                            nc.tensor.matmul(
                                hp[:], f1t[:, kc, :], xlnT[:, kc, :],
                                start=(kc == 0), stop=(kc == KC - 1))
                        nc.scalar.activation(hT[:, mh, :], hp[:], AF.Gelu,
                                             bias=ff1b_t[:, mh:mh + 1])

                    ff2b_t = sm.tile([128, D], F32, name="ff2b")
                    bcast_load(ff2b_t[:], t["ff2b"][l])
                    ln2w_t = sm.tile([128, D], F32, name="ln2w")
                    ln2b_t = sm.tile([128, D], F32, name="ln2b")
                    bcast_load(ln2w_t[:], t["ln2w"][l])
                    bcast_load(ln2b_t[:], t["ln2b"][l])
                    y2p = [[py2.tile([128, 512], F32, name=f"y2_{m}{n}")
                            for n in range(2)] for m in range(2)]
                    for g in range(4):
                        f2t = st.tile([128, KC, D], BF16, name="f2t")
                        nc.sync.dma_start(f2t[:], t["ff2"][l, g])
                        for kc in range(KC):
                            gk = g * KC + kc
                            for m in range(2):
                                for n in range(2):
                                    nc.tensor.matmul(
                                        y2p[m][n][:],
                                        hT[:, gk, m * 128:(m + 1) * 128],
                                        f2t[:, kc, n * 512:(n + 1) * 512],
                                        start=(gk == 0),
                                        stop=(gk == FF // 128 - 1))
                    for m in range(2):
                        for n in range(2):
                            nsl = slice(n * 512, (n + 1) * 512)
                            nc.vector.tensor_add(z[m][:, nsl], y2p[m][n][:],
                                                 xln[m][:, nsl])
                        nc.vector.tensor_add(z[m][:], z[m][:], ff2b_t[:])
                        layer_norm(x_own[m], z[m], ln2w_t, ln2b_t, sm,
                                   nc.gpsimd if m == 1 else nc.vector)

                # ---- next xT AllGather (two halves) ----
                with nc.named_scope(f"L{l}.ag"):
                    with tc.tile_pool(name="ph_tx", bufs=1) as pp, \
                         tc.tile_pool(name="ps_wa", bufs=1,
                                      space="PSUM") as pwm, \
                         tc.tile_pool(name="ps_ta", bufs=2,
                                      space="PSUM") as ptp:
                        gate = pp.tile([128, 128], BF16, name="wgate")
                        nc.vector.tensor_copy(gate[:], z[0][:, 0:128])
                        warm_chain(pwm, gate[:], 65)
                        gather_xt(x_own, l + 1, pp, ptp, pwm,
                                  200 if l == n_layers - 1 else 130)

        if debug_x:
            for m in range(2):
                nc.sync.dma_start(
                    t["dbg_x"][m * 128:(m + 1) * 128, :], x_own[m][:])

        # ---------------- vocab projection ----------------
        with nc.named_scope("vocab"), \
             tc.tile_pool(name="ph_voc", bufs=1) as pp, \
             tc.tile_pool(name="voc_st", bufs=3) as st, \
             tc.tile_pool(name="voc_sm", bufs=4) as sm, \
             tc.tile_pool(name="ps_voc", bufs=4, space="PSUM") as pv:
            XT = pp.tile([128, NC, KC, 256], BF16, name="XTf")
            for rb in range(NC):
                nc.gpsimd.dma_start(XT[:, rb], t["xt_out_b"][n_layers][rb])
            for vc in range(NVC):
                vlen = VCL if vc == NVC - 1 else 512
                # compute width: pad last chunk to 128 (not 512) cols
                cw = 128 if vc == NVC - 1 else 512
                voff = vc * 512
                wv_t = st.tile([128, KC, 512], BF16, name="wvoc")
                nc.sync.dma_start(wv_t[:, :, 0:cw], t["outw"][vc][:, :, 0:cw])
                ob_t = sm.tile([128, 512], F32, name="ob")
                bcast_load(ob_t[:, 0:cw], t["outb"][vc][0:cw])
                for sc in range(16):
                    rb, half = sc // 2, sc % 2
                    lpp = pv.tile([128, 512], F32, name="log_ps")
                    for kc in range(KC):
                        nc.tensor.matmul(
                            lpp[:, 0:cw],
                            XT[:, rb, kc, half * 128:(half + 1) * 128],
                            wv_t[:, kc, 0:cw],
                            start=(kc == 0), stop=(kc == KC - 1))
                    lo = sm.tile([128, 512], F32, name="log_sb")
                    nc.vector.tensor_add(lo[:, 0:cw], lpp[:, 0:cw],
                                         ob_t[:, 0:cw])
                    nc.scalar.dma_start(
                        t["logits"][sc * 128:(sc + 1) * 128,
                                    voff:voff + vlen],
                        lo[:, 0:vlen])


def _prepare_in_maps(inputs):
    ids = np.asarray(inputs["input_ids"]).reshape(S).astype(np.int32)
    cos, sin = _np_rope_tables()          # [S, DK]
    # head-duplicated transposed tables; sin with the rotate-half sign
    # folded in: out = q*cos + shuffle(q)*sinm
    cosT = np.ascontiguousarray(np.concatenate([cos.T, cos.T], 0))  # [128, S]
    sinm = sin.T.copy()                   # [DK, S]
    sinm[:32] = -sinm[:32]
    sinmT = np.ascontiguousarray(np.concatenate([sinm, sinm], 0))
    masks = _diag_masks()
    f = np.float32

    def bf(x):
        return np.ascontiguousarray(np.asarray(x, np.float32)).astype(BF_NP)

    def tile_w(w):
        # [D, M] -> [128(p), KC, M] per-partition contiguous
        d, m = w.shape
        return np.ascontiguousarray(
            np.asarray(w, f).reshape(KC, 128, m).transpose(1, 0, 2))

    wq_full = np.asarray(inputs["wq"], f)
    wk_full = np.asarray(inputs["wk"], f)
    wv_full = np.asarray(inputs["wv"], f)
    wo_full = np.asarray(inputs["wo_w"], f)
    ff1_full = np.asarray(inputs["ff1_w"], f)
    ff2_full = np.asarray(inputs["ff2_w"], f)
    outw_full = np.asarray(inputs["out_w"], f)
    outb_full = np.asarray(inputs["out_b"], f)

    # common (unsharded) tensors prepared once
    wo_t = bf(np.stack([tile_w(wo_full[l]) for l in range(L)]))
    # ff1: [L, mh, 128, KC, 128]
    ff1_t = np.stack([
        np.asarray(ff1_full[l], f).reshape(KC, 128, FF // 128, 128)
        .transpose(2, 1, 0, 3)
        for l in range(L)])
    ff1_t = bf(ff1_t)
    # ff1b: [L, 128, 32] with ff1b[l, p, m] = ff1_b[l, m*128+p]
    ff1b_t = np.ascontiguousarray(
        np.asarray(inputs["ff1_b"], f).reshape(L, FF // 128, 128)
        .transpose(0, 2, 1))
    # ff2: [L, g, 128, KC(within g), D]
    ff2_t = np.stack([
        np.asarray(ff2_full[l], f).reshape(4, KC, 128, D).transpose(0, 2, 1, 3)
        for l in range(L)])
    ff2_t = bf(ff2_t)
    temb_bf = bf(inputs["token_emb"])
    dmask_bf = masks.astype(BF_NP)
    cosT_bf = cosT.astype(BF_NP)
    sinmT_bf = sinmT.astype(BF_NP)

    in_maps = []
    for r in range(NC):
        hsl = slice(r * DHL, (r + 1) * DHL)
        # out_w shard -> [NVC, 128, KC, 512] padded
        ow = np.zeros((NVC, 128, KC, 512), np.float32)
        owr = outw_full[:, r * VL:(r + 1) * VL]          # [D, VL]
        owr_t = owr.reshape(KC, 128, VL).transpose(1, 0, 2)  # [128, KC, VL]
        ow[:, :, :, :] = 0.0
        for vc in range(NVC):
            vlen = VCL if vc == NVC - 1 else 512
            ow[vc, :, :, 0:vlen] = owr_t[:, :, vc * 512:vc * 512 + vlen]
        ob = np.zeros((NVC, 512), np.float32)
        obr = outb_full[r * VL:(r + 1) * VL]
        for vc in range(NVC):
            vlen = VCL if vc == NVC - 1 else 512
            ob[vc, 0:vlen] = obr[vc * 512:vc * 512 + vlen]

        im = {
            "ids": np.ascontiguousarray(
                ids[r * SL:(r + 1) * SL].reshape(2, 128, 1)),
            "token_emb": temb_bf,
            "pos_emb": np.ascontiguousarray(
                np.asarray(inputs["pos_emb"], f)[r * SL:(r + 1) * SL]),
            "wq": bf(np.stack([tile_w(wq_full[l][:, hsl])
                               for l in range(L)])),
            "wk": bf(np.stack([tile_w(wk_full[l][:, hsl])
                               for l in range(L)])),
            "wv": bf(np.stack([tile_w(wv_full[l][:, hsl])
                               for l in range(L)])),
            "wo_w": wo_t,
            "wo_b": np.asarray(inputs["wo_b"], f),
            "ln1_w": np.asarray(inputs["ln1_w"], f),
            "ln1_b": np.asarray(inputs["ln1_b"], f),
            "ln2_w": np.asarray(inputs["ln2_w"], f),
            "ln2_b": np.asarray(inputs["ln2_b"], f),
            "ff1_w": ff1_t,
            "ff1_b": ff1b_t,
            "ff2_w": ff2_t,
            "ff2_b": np.asarray(inputs["ff2_b"], f),
            "out_w": ow.astype(BF_NP),
            "out_b": ob,
            "cosT": cosT_bf,
            "sinmT": sinmT_bf,
            "dmask": dmask_bf,
        }
        in_maps.append(im)
    return in_maps


def run(inputs, n_layers=L, debug_x=False, **kw):
    key = (n_layers, debug_x)
    if key not in _CACHE:
        _CACHE[key] = build_program(n_layers, debug_x)
    nc = _CACHE[key]
    in_maps = _prepare_in_maps(inputs)
    res = run_bass_kernel_spmd(nc, in_maps, list(range(NC)), **kw)
    return res


def kernel(**inputs):
    res = run(inputs)
    logits = np.concatenate([res.results[r]["logits"] for r in range(NC)],
                            axis=1)
    return logits.reshape(B, S, V)


# revision 35
# speedup vs baseline: 1.0637x; 1.0637x over previous
"""Trainium2 Bass kernel for a 4-layer dense transformer (B=1, S=2048, D=1024,
H=16, DK=64, FF=4096, V=50000) distributed over 8 NeuronCores.

Sharding:
  - Attention: tensor-parallel over heads (2 heads/core), full sequence.
  - LayerNorm / FFN / residual: sequence-parallel (256 rows/core), full width.
  - Vocab projection: sharded over vocab (6250 cols/core).
  - Glue per layer: AllGather of x^T (for QKV inputs) and AllToAll of the
    normalized ctx^T (delivers every head's dims for the core's own rows).
    One final AllGather before the vocab matmul.

v2: all matmul operands in bf16 (FWL weight loads, half DMA/collective
traffic), host-side weight re-tiling for contiguous >=2KB DMA descriptors,
two-head-batched softmax exp, LN rstd via ln+exp (stays in the exp ACT
table set), RoPE in bf16 with DMA-based rotate-half shuffle and sign-folded
sin tables. Residual/LN/softmax accumulation stays fp32.
"""
import sys

if "/opt/trn_rl_repo" not in sys.path:
    sys.path.insert(0, "/opt/trn_rl_repo")

import contextlib

import ml_dtypes
import numpy as np

import concourse.bass as bass
import concourse.tile as tile
from concourse import bacc, mybir
from concourse.bass_utils import run_bass_kernel_spmd
from concourse.masks import make_identity

F32 = mybir.dt.float32
BF16 = mybir.dt.bfloat16
I32 = mybir.dt.int32
AF = mybir.ActivationFunctionType
BF_NP = ml_dtypes.bfloat16

NC = 8                    # cores
B, S, D, H, DK, FF, V, L = 1, 2048, 1024, 16, 64, 4096, 50000, 4
EPS = 1e-5
SCALE = 1.0 / np.sqrt(DK)
HL = H // NC              # heads per core = 2
DHL = HL * DK             # local head dims = 128
SL = S // NC              # rows per core = 256
VL = V // NC              # vocab per core = 6250
KC = D // 128             # contraction chunks over D = 8
NVC = 13                  # vocab chunks of 512 (last is 106 real)
VCL = 106                 # real cols in last chunk

_CACHE = {}


def _np_rope_tables():
    inv_freq = 1.0 / (10000.0 ** (np.arange(0, DK, 2, dtype=np.float32) / DK))
    t = np.arange(S, dtype=np.float32)
    freqs = np.outer(t, inv_freq)                 # [S, DK/2]
    emb = np.concatenate([freqs, freqs], -1)      # [S, DK]
    return np.cos(emb), np.sin(emb)


def _diag_masks():
    # Triangle mask for the 128x128 block straddling the diagonal: within
    # cols [128d, 128d+128) of a diag chunk, allowed iff local col j >= k.
    # Identical for every d; stored per-head-duplicated: [128, 4, 2, 128].
    masks = np.zeros((128, 4, 2, 128), np.float32)
    k = np.arange(128)[:, None]
    j = np.arange(128)[None, :]
    m = (j >= k).astype(np.float32)
    for d in range(4):
        masks[:, d, 0, :] = m
        masks[:, d, 1, :] = m
    return masks


def build_program(n_layers=L, debug_x=False):
    nc = bacc.Bacc("TRN2", target_bir_lowering=False, debug=False,
                   num_devices=NC)

    t = {}
    t["ids"] = nc.dram_tensor("ids", [2, 128, 1], I32, kind="ExternalInput")
    t["temb"] = nc.dram_tensor("token_emb", [V, D], BF16, kind="ExternalInput")
    t["pemb"] = nc.dram_tensor("pos_emb", [SL, D], F32, kind="ExternalInput")
    # weights pre-tiled on host: per-partition contiguous
    t["wq"] = nc.dram_tensor("wq", [L, 128, KC, DHL], BF16,
                             kind="ExternalInput")
    t["wk"] = nc.dram_tensor("wk", [L, 128, KC, DHL], BF16,
                             kind="ExternalInput")
    t["wv"] = nc.dram_tensor("wv", [L, 128, KC, DHL], BF16,
                             kind="ExternalInput")
    t["wo"] = nc.dram_tensor("wo_w", [L, 128, KC, D], BF16,
                             kind="ExternalInput")
    t["wob"] = nc.dram_tensor("wo_b", [L, D], F32, kind="ExternalInput")
    t["ln1w"] = nc.dram_tensor("ln1_w", [L, D], F32, kind="ExternalInput")
    t["ln1b"] = nc.dram_tensor("ln1_b", [L, D], F32, kind="ExternalInput")
    t["ln2w"] = nc.dram_tensor("ln2_w", [L, D], F32, kind="ExternalInput")
    t["ln2b"] = nc.dram_tensor("ln2_b", [L, D], F32, kind="ExternalInput")
    t["ff1"] = nc.dram_tensor("ff1_w", [L, FF // 128, 128, KC, 128], BF16,
                              kind="ExternalInput")
    t["ff1b"] = nc.dram_tensor("ff1_b", [L, 128, FF // 128], F32,
                               kind="ExternalInput")
    t["ff2"] = nc.dram_tensor("ff2_w", [L, 4, 128, KC, D], BF16,
                              kind="ExternalInput")
    t["ff2b"] = nc.dram_tensor("ff2_b", [L, D], F32, kind="ExternalInput")
    t["outw"] = nc.dram_tensor("out_w", [NVC, 128, KC, 512], BF16,
                               kind="ExternalInput")
    t["outb"] = nc.dram_tensor("out_b", [NVC, 512], F32, kind="ExternalInput")
    t["cos"] = nc.dram_tensor("cosT", [128, S], BF16, kind="ExternalInput")
    t["sinm"] = nc.dram_tensor("sinmT", [128, S], BF16, kind="ExternalInput")
    t["dmask"] = nc.dram_tensor("dmask", [128, 4, 2, 128], BF16,
                                kind="ExternalInput")

    t["logits"] = nc.dram_tensor("logits", [S, VL], F32, kind="ExternalOutput")
    if debug_x:
        t["dbg_x"] = nc.dram_tensor("dbg_x", [SL, D], F32,
                                    kind="ExternalOutput")

    # collective bounce buffers (bf16), split for compute/collective overlap
    t["comm_in"] = nc.dram_tensor("comm_in", [16], BF16)
    t["comm_out"] = nc.dram_tensor("comm_out", [NC, 16], BF16,
                                   addr_space="Shared")
    t["xt_in_b"] = [nc.dram_tensor(f"xt_in_{l}", [128, KC, SL], BF16)
                    for l in range(n_layers + 1)]
    t["xt_out_b"] = [nc.dram_tensor(f"xt_out_{l}", [NC, 128, KC, SL], BF16,
                                    addr_space="Shared")
                     for l in range(n_layers + 1)]
    t["cx_in_b"] = [nc.dram_tensor(f"cx_in_{l}", [NC, 128, SL], BF16)
                    for l in range(n_layers)]
    t["cx_out_b"] = [nc.dram_tensor(f"cx_out_{l}", [NC, 128, SL], BF16)
                     for l in range(n_layers)]

    with tile.TileContext(nc) as tc:
        _build(nc, tc, t, n_layers, debug_x)
    nc.compile()
    return nc


def _build(nc, tc, t, n_layers, debug_x):
    rg = [list(range(NC))]
    es = contextlib.ExitStack()
    with es:
        const = es.enter_context(tc.tile_pool(name="const", bufs=1))
        glob = es.enter_context(tc.tile_pool(name="glob", bufs=1))

        # ---------------- constants ----------------
        ident = const.tile([128, 128], F32)
        make_identity(nc, ident[:])
        cos_t = const.tile([128, S], BF16)
        sinm_t = const.tile([128, S], BF16)
        nc.sync.dma_start(cos_t[:], t["cos"][:, :])
        nc.sync.dma_start(sinm_t[:], t["sinm"][:, :])
        dmask_t = const.tile([128, 4, 2, 128], BF16)
        nc.sync.dma_start(dmask_t[:], t["dmask"][:, :, :, :])
        ones_t = const.tile([128, 16], BF16)
        nc.vector.memset(ones_t[:], 1.0)
        eps_t = const.tile([128, 1], F32)
        nc.vector.memset(eps_t[:], EPS)
        warm_src = const.tile([128, 512], BF16)
        nc.vector.memset(warm_src[:], 0.001)

        def warm_chain(psum_pool, dep_ap, n):
            """Keep the PE busy with dummy matmuls while a collective runs.

            dep_ap gates the chain start (SBUF tile written just before the
            gap); WAW on the psum tile serializes the chain so it spans the
            idle window instead of racing ahead. Keeps the HAM activity
            monitor from dropping the PE clock to 1.2GHz."""
            wp = psum_pool.tile([128, 512], F32, name="warm")
            for _ in range(n):
                nc.tensor.matmul(wp[:], dep_ap, warm_src[:],
                                 start=True, stop=True)

        def bcast_load(dst, src_1d):
            """DMA a [N] DRAM vector into a [P, N] tile, replicated."""
            p = dst.shape[0]
            ap = bass.AP(tensor=src_1d.tensor, offset=src_1d.offset,
                         ap=[[0, p]] + src_1d.ap)
            nc.sync.dma_start(dst, ap)

        # x_own[m]: [128, 1024] f32, own rows (m=0: rows 0..127 of the
        # core's 256; m=1: rows 128..255)
        x_own = [glob.tile([128, D], F32, name=f"x_own{m}") for m in range(2)]

        def gather_xt(src_tiles, lx, pool, tp_pool, warm_pool=None,
                      warm_n=0):
            """src [2][128, 1024] f32 -> xt bounce (bf16, via PE
            transpose) followed by its AllGather."""
            xt_sb = pool.tile([128, KC, 256], BF16, name="xt_sb")
            for kc in range(KC):
                for m in range(2):
                    tp = tp_pool.tile([128, 128], F32, name="tp_ps")
                    nc.tensor.transpose(
                        tp[:], src_tiles[m][:, kc * 128:(kc + 1) * 128],
                        ident[:])
                    nc.vector.tensor_copy(
                        xt_sb[:, kc, m * 128:(m + 1) * 128], tp[:])
            nc.sync.dma_start(t["xt_in_b"][lx][:, :, :], xt_sb[:])
            nc.gpsimd.collective_compute(
                "AllGather", mybir.AluOpType.bypass, replica_groups=rg,
                ins=[t["xt_in_b"][lx][:, :, :]],
                outs=[t["xt_out_b"][lx][:, :, :, :]])
            if warm_n:
                warm_chain(warm_pool, xt_sb[:, 0, 0:128], warm_n)

        def layer_norm(dst, src, w_t, b_t, small, eng=None):
            eng = eng or nc.vector
            st = small.tile([128, 2, 6], F32, name="bn_st")
            mv = small.tile([128, 2], F32, name="bn_mv")
            for g in range(2):
                nc.vector.bn_stats(st[:, g, :],
                                   src[:, g * 512:(g + 1) * 512])
            nc.vector.bn_aggr(mv[:], st[:])
            rstd = small.tile([128, 1], F32, name="rstd")
            nc.scalar.activation(rstd[:], mv[:, 1:2], AF.Sqrt, bias=eps_t[:])
            nc.vector.reciprocal(rstd[:], rstd[:])
            # apply in column halves so downstream per-column consumers
            # (PE transposes) can start on cols 0-511 early
            for g in range(2):
                gs = slice(g * 512, (g + 1) * 512)
                eng.tensor_scalar(
                    out=dst[:, gs], in0=src[:, gs], scalar1=mv[:, 0:1],
                    scalar2=rstd[:], op0=mybir.AluOpType.subtract,
                    op1=mybir.AluOpType.mult)
                eng.tensor_mul(dst[:, gs], dst[:, gs], w_t[:, gs])
                eng.tensor_add(dst[:, gs], dst[:, gs], b_t[:, gs])

        # ---------------- embedding ----------------
        with nc.named_scope("embed"):
            # fire a tiny collective first: the ~100us cross-core comm-init
            # barrier runs while the embedding computes
            nc.vector.memset(ones_t[:, 0:1], 1.0)
            nc.sync.dma_start(t["comm_in"][:], ones_t[0:1, 0:16])
            nc.gpsimd.collective_compute(
                "AllGather", mybir.AluOpType.bypass, replica_groups=rg,
                ins=[t["comm_in"][:]], outs=[t["comm_out"][:, :]])
            with tc.tile_pool(name="emb", bufs=2) as emb:
                for m in range(2):
                    idx_t = emb.tile([128, 1], I32, name="idx")
                    nc.sync.dma_start(idx_t[:], t["ids"][m, :, :])
                    gat = emb.tile([128, D], BF16, name="gat")
                    nc.gpsimd.indirect_dma_start(
                        out=gat[:], out_offset=None, in_=t["temb"][:, :],
                        in_offset=bass.IndirectOffsetOnAxis(ap=idx_t[:, :1],
                                                            axis=0))
                    pos_t = emb.tile([128, D], F32, name="pos")
                    nc.sync.dma_start(pos_t[:],
                                      t["pemb"][m * 128:(m + 1) * 128, :])
                    nc.vector.tensor_add(x_own[m][:], gat[:], pos_t[:])
                with tc.tile_pool(name="ps_we", bufs=1, space="PSUM") as pwm, \
                     tc.tile_pool(name="ps_te", bufs=2, space="PSUM") as ptp:
                    gather_xt(x_own, 0, emb, ptp, pwm, 160)

        # ---------------- layers ----------------
        for l in range(n_layers):
            with tc.tile_pool(name=f"layer{l}", bufs=1) as lp:
                qTr = lp.tile([128, S], BF16, name="qTr")
                kTr = lp.tile([128, S], BF16, name="kTr")
                v_aug = [lp.tile([128, 16, 65], BF16, name=f"vaug{h}")
                         for h in range(HL)]
                ctxc = lp.tile([128, S], BF16, name="ctxc")
                xln = [lp.tile([128, D], F32, name=f"xln{m}")
                       for m in range(2)]
                z = [lp.tile([128, D], F32, name=f"zz{m}") for m in range(2)]
                # prefetch out-proj weights/params at layer start
                wo_sb = lp.tile([128, KC, D], BF16, name="wo_sb")
                nc.sync.dma_start(wo_sb[:], t["wo"][l])
                wob_t = lp.tile([128, D], F32, name="wob")
                bcast_load(wob_t[:], t["wob"][l])
                ln1w_t = lp.tile([128, D], F32, name="ln1w")
                ln1b_t = lp.tile([128, D], F32, name="ln1b")
                bcast_load(ln1w_t[:], t["ln1w"][l])
                bcast_load(ln1b_t[:], t["ln1b"][l])

                # ---- phase 1: QKV (+ inline RoPE per 512-col chunk) ----
                with nc.named_scope(f"L{l}.qkv"), \
                     tc.tile_pool(name="ph_qkv", bufs=2) as pp, \
                     tc.tile_pool(name="qkv_qk", bufs=1) as qk, \
                     tc.tile_pool(name="ps_qkv", bufs=3, space="PSUM") as pq:
                    wq_sb = pp.tile([128, KC, DHL], BF16, name="wq_sb")
                    wk_sb = pp.tile([128, KC, DHL], BF16, name="wk_sb")
                    wv_sb = pp.tile([128, KC, DHL], BF16, name="wv_sb")
                    nc.sync.dma_start(wq_sb[:], t["wq"][l])
                    nc.sync.dma_start(wk_sb[:], t["wk"][l])
                    nc.sync.dma_start(wv_sb[:], t["wv"][l])
                    for h in range(HL):
                        nc.vector.tensor_copy(v_aug[h][:, :, 64], ones_t[:])
                    qT = qk.tile([128, S], BF16, name="qT")
                    kT = qk.tile([128, S], BF16, name="kT")
                    for rb in range(NC):
                        ssl = slice(rb * 256, (rb + 1) * 256)
                        xt_c = pp.tile([128, KC, 256], BF16, name="xt_c")
                        nc.gpsimd.dma_start(xt_c[:], t["xt_out_b"][l][rb])
                        for ci, (w_sb, dstT) in enumerate(
                                ((wk_sb, kT), (wq_sb, qT))):
                            pt = pq.tile([128, 256], F32, name="qk_ps")
                            for kc in range(KC):
                                nc.tensor.matmul(
                                    pt[:], w_sb[:, kc, :],
                                    xt_c[:, kc, :],
                                    start=(kc == 0), stop=(kc == KC - 1))
                            if ci == 0:
                                nc.scalar.copy(dstT[:, ssl], pt[:])
                            else:
                                nc.vector.tensor_copy(dstT[:, ssl], pt[:])
                        for half in range(2):
                            sc = rb * 2 + half
                            pt = pq.tile([128, DHL], F32, name="v_ps")
                            for kc in range(KC):
                                nc.tensor.matmul(
                                    pt[:],
                                    xt_c[:, kc, half * 128:(half + 1) * 128],
                                    wv_sb[:, kc, :],
                                    start=(kc == 0), stop=(kc == KC - 1))
                            for h in range(HL):
                                nc.scalar.copy(
                                    v_aug[h][:, sc, 0:64],
                                    pt[:, h * 64:(h + 1) * 64])
                        # RoPE on the completed 512-col chunk (rb odd)
                        if rb % 2 == 1:
                            cb = rb // 2
                            csl = slice(cb * 512, (cb + 1) * 512)
                            for srcT, dstT in ((kT, kTr), (qT, qTr)):
                                rh = pp.tile([128, 512], BF16, name="rope_rh")
                                for h in range(HL):
                                    lo = h * 64
                                    hi = h * 64 + 32
                                    nc.sync.dma_start(rh[lo:hi, 0:512],
                                                      srcT[hi:hi + 32, csl])
                                    nc.sync.dma_start(rh[hi:hi + 32, 0:512],
                                                      srcT[lo:hi, csl])
                                nc.gpsimd.tensor_mul(
                                    rh[:], rh[:], sinm_t[:, csl])
                                tmp = pp.tile([128, 512], BF16, name="rope_t2")
                                nc.vector.tensor_mul(
                                    tmp[:], srcT[:, csl], cos_t[:, csl])
                                nc.vector.tensor_add(dstT[:, csl], tmp[:],
                                                     rh[:])

                # ---- phase 2: attention ----
                with nc.named_scope(f"L{l}.att"), \
                     tc.tile_pool(name="ph_att", bufs=3) as pp, \
                     tc.tile_pool(name="ps_sc", bufs=2, space="PSUM") as psc, \
                     tc.tile_pool(name="ps_ct", bufs=2, space="PSUM") as pct, \
                     tc.tile_pool(name="att_sm", bufs=3) as sm:
                    for qb in range(4):
                        nkc = (qb + 1) * 4
                        qsl = slice(qb * 512, (qb + 1) * 512)
                        ct_ps = [pct.tile([65, 512], F32, name=f"ct_ps{h}")
                                 for h in range(HL)]
                        for kc in range(nkc):
                            ksl = slice(kc * 128, (kc + 1) * 128)
                            sc_ps = psc.tile([128, 2, 512], F32, name="sc_ps")
                            for h in range(HL):
                                nc.tensor.matmul(
                                    sc_ps[:, h, :],
                                    kTr[h * 64:(h + 1) * 64, ksl],
                                    qTr[h * 64:(h + 1) * 64, qsl],
                                    start=True, stop=True,
                                    tile_position=(64 * h, 0))
                            et = pp.tile([128, 2, 512], BF16, name="exp")
                            d = kc - qb * 4
                            # diag chunk d: cols < 128*d are fully masked —
                            # skip their exp (the mask mult zeroes stale
                            # data; every et slot sees a full-range write
                            # first, so stale values are finite).
                            c0 = 128 * d if d > 0 else 0
                            nc.scalar.activation(et[:, :, c0:],
                                                 sc_ps[:, :, c0:],
                                                 AF.Exp, scale=float(SCALE))
                            if d > 0:
                                nc.gpsimd.memset(et[:, :, 0:c0], 0.0)
                            if d >= 0:
                                nc.vector.tensor_mul(
                                    et[:, :, c0:c0 + 128], et[:, :, c0:c0 + 128],
                                    dmask_t[:, d, :, :])
                            for h in range(HL):
                                nc.tensor.matmul(
                                    ct_ps[h][:], v_aug[h][:, kc, :],
                                    et[:, h, :],
                                    start=(kc == 0), stop=(kc == nkc - 1))
                        for h in range(HL):
                            den = sm.tile([1, 512], F32, name="den")
                            nc.vector.tensor_copy(den[:], ct_ps[h][64:65, :])
                            rec = sm.tile([1, 512], F32, name="rec")
                            nc.vector.reciprocal_approx_fast(rec[:], den[:])
                            bc = sm.tile([64, 512], F32, name="bc")
                            nc.gpsimd.partition_broadcast(bc[:], rec[:])
                            nc.vector.tensor_mul(
                                ctxc[h * 64:(h + 1) * 64, qsl],
                                ct_ps[h][0:64, :], bc[:])

                # ---- AllToAll ctx: shard j = ctxc[:, j*256:(j+1)*256] ----
                with nc.named_scope(f"L{l}.a2a"):
                    for j in range(NC):
                        nc.sync.dma_start(t["cx_in_b"][l][j],
                                          ctxc[:, j * SL:(j + 1) * SL])
                    nc.gpsimd.collective_compute(
                        "AllToAll", mybir.AluOpType.bypass, replica_groups=rg,
                        ins=[t["cx_in_b"][l][:, :, :]],
                        outs=[t["cx_out_b"][l][:, :, :]])

                # ---- phase 3: out-proj + LN1 ----
                with nc.named_scope(f"L{l}.wo"), \
                     tc.tile_pool(name="ph_wo", bufs=1) as pp, \
                     tc.tile_pool(name="wo_sm", bufs=3) as sm, \
                     tc.tile_pool(name="ps_wo", bufs=2, space="PSUM") as pw:
                    warm_chain(pw, ctxc[:, S - 128:S], 90)
                    ctxT = pp.tile([128, KC, 256], BF16, name="ctxT")
                    for rb in range(NC):
                        nc.gpsimd.dma_start(ctxT[:, rb, :],
                                            t["cx_out_b"][l][rb])
                    for m in range(2):
                        for n in range(2):
                            yp = pw.tile([128, 512], F32, name="y_ps")
                            for kc in range(KC):
                                nc.tensor.matmul(
                                    yp[:], ctxT[:, kc, m * 128:(m + 1) * 128],
                                    wo_sb[:, kc, n * 512:(n + 1) * 512],
                                    start=(kc == 0), stop=(kc == KC - 1))
                            nsl = slice(n * 512, (n + 1) * 512)
                            nc.vector.tensor_add(z[m][:, nsl], yp[:],
                                                 x_own[m][:, nsl])
                        nc.vector.tensor_add(z[m][:], z[m][:], wob_t[:])
                        layer_norm(xln[m], z[m], ln1w_t, ln1b_t, sm,
                                   nc.gpsimd if m == 1 else nc.vector)

                # ---- phase 4: FFN + LN2 ----
                with nc.named_scope(f"L{l}.ffn"), \
                     tc.tile_pool(name="ph_ff", bufs=1) as pp, \
                     tc.tile_pool(name="ff_st", bufs=3) as st, \
                     tc.tile_pool(name="ff_sm", bufs=3) as sm, \
                     tc.tile_pool(name="ps_h", bufs=2, space="PSUM") as ph, \
                     tc.tile_pool(name="ps_tf", bufs=2, space="PSUM") as ptp, \
                     tc.tile_pool(name="ps_y2", bufs=1, space="PSUM") as py2:
                    xlnT = pp.tile([128, KC, 256], BF16, name="xlnT")
                    for m in range(2):
                        for kc in range(KC):
                            tp = ptp.tile([128, 128], F32, name="tp_ps")
                            nc.tensor.transpose(
                                tp[:], xln[m][:, kc * 128:(kc + 1) * 128],
                                ident[:])
                            nc.vector.tensor_copy(
                                xlnT[:, kc, m * 128:(m + 1) * 128], tp[:])
                    ff1b_t = sm.tile([128, FF // 128], F32, name="ff1b")
                    nc.sync.dma_start(ff1b_t[:], t["ff1b"][l])
                    hT = pp.tile([128, FF // 128, 256], BF16, name="hT")
                    for mh in range(FF // 128):
                        f1t = st.tile([128, KC, 128], BF16, name="f1t")
                        nc.sync.dma_start(f1t[:], t["ff1"][l, mh])
                        hp = ph.tile([128, 256], F32, name="h_ps")
                        for kc in range(KC):
                            nc.tensor.matmul(
                                hp[:], f1t[:, kc, :], xlnT[:, kc, :],
                                start=(kc == 0), stop=(kc == KC - 1))
                        nc.scalar.activation(hT[:, mh, :], hp[:], AF.Gelu,
                                             bias=ff1b_t[:, mh:mh + 1])

                    ff2b_t = sm.tile([128, D], F32, name="ff2b")
                    bcast_load(ff2b_t[:], t["ff2b"][l])
                    ln2w_t = sm.tile([128, D], F32, name="ln2w")
                    ln2b_t = sm.tile([128, D], F32, name="ln2b")
                    bcast_load(ln2w_t[:], t["ln2w"][l])
                    bcast_load(ln2b_t[:], t["ln2b"][l])
                    y2p = [[py2.tile([128, 512], F32, name=f"y2_{m}{n}")
                            for n in range(2)] for m in range(2)]
                    for g in range(4):
                        f2t = st.tile([128, KC, D], BF16, name="f2t")
                        nc.sync.dma_start(f2t[:], t["ff2"][l, g])
                        for kc in range(KC):
                            gk = g * KC + kc
                            for m in range(2):
                                for n in range(2):
                                    nc.tensor.matmul(
                                        y2p[m][n][:],
                                        hT[:, gk, m * 128:(m + 1) * 128],
                                        f2t[:, kc, n * 512:(n + 1) * 512],
                                        start=(gk == 0),
                                        stop=(gk == FF // 128 - 1))
                    for m in range(2):
                        for n in range(2):
                            nsl = slice(n * 512, (n + 1) * 512)
                            nc.vector.tensor_add(z[m][:, nsl], y2p[m][n][:],
                                                 xln[m][:, nsl])
                        nc.vector.tensor_add(z[m][:], z[m][:], ff2b_t[:])
                        layer_norm(x_own[m], z[m], ln2w_t, ln2b_t, sm,
                                   nc.gpsimd if m == 1 else nc.vector)

                # ---- next xT AllGather (two halves) ----
                with nc.named_scope(f"L{l}.ag"):
                    with tc.tile_pool(name="ph_tx", bufs=1) as pp, \
                         tc.tile_pool(name="ps_wa", bufs=1,
                                      space="PSUM") as pwm, \
                         tc.tile_pool(name="ps_ta", bufs=2,
                                      space="PSUM") as ptp:
                        gate = pp.tile([128, 128], BF16, name="wgate")
                        nc.vector.tensor_copy(gate[:], z[0][:, 0:128])
                        warm_chain(pwm, gate[:], 45)
                        gather_xt(x_own, l + 1, pp, ptp, pwm,
                                  200 if l == n_layers - 1 else 130)

        if debug_x:
            for m in range(2):
                nc.sync.dma_start(
                    t["dbg_x"][m * 128:(m + 1) * 128, :], x_own[m][:])

        # ---------------- vocab projection ----------------
        with nc.named_scope("vocab"), \
             tc.tile_pool(name="ph_voc", bufs=1) as pp, \
             tc.tile_pool(name="voc_st", bufs=3) as st, \
             tc.tile_pool(name="voc_sm", bufs=4) as sm, \
             tc.tile_pool(name="ps_voc", bufs=4, space="PSUM") as pv:
            XT = pp.tile([128, NC, KC, 256], BF16, name="XTf")
            for rb in range(NC):
                nc.gpsimd.dma_start(XT[:, rb], t["xt_out_b"][n_layers][rb])
            for vc in range(NVC):
                vlen = VCL if vc == NVC - 1 else 512
                # compute width: pad last chunk to 128 (not 512) cols
                cw = 128 if vc == NVC - 1 else 512
                voff = vc * 512
                wv_t = st.tile([128, KC, 512], BF16, name="wvoc")
                nc.sync.dma_start(wv_t[:, :, 0:cw], t["outw"][vc][:, :, 0:cw])
                ob_t = sm.tile([128, 512], F32, name="ob")
                bcast_load(ob_t[:, 0:cw], t["outb"][vc][0:cw])
                for sc in range(16):
                    rb, half = sc // 2, sc % 2
                    lpp = pv.tile([128, 512], F32, name="log_ps")
                    for kc in range(KC):
                        nc.tensor.matmul(
                            lpp[:, 0:cw],
                            XT[:, rb, kc, half * 128:(half + 1) * 128],
                            wv_t[:, kc, 0:cw],
                            start=(kc == 0), stop=(kc == KC - 1))
                    lo = sm.tile([128, 512], F32, name="log_sb")
                    nc.vector.tensor_add(lo[:, 0:cw], lpp[:, 0:cw],
                                         ob_t[:, 0:cw])
                    nc.scalar.dma_start(
                        t["logits"][sc * 128:(sc + 1) * 128,
                                    voff:voff + vlen],
                        lo[:, 0:vlen])


def _prepare_in_maps(inputs):
    ids = np.asarray(inputs["input_ids"]).reshape(S).astype(np.int32)
    cos, sin = _np_rope_tables()          # [S, DK]
    # head-duplicated transposed tables; sin with the rotate-half sign
    # folded in: out = q*cos + shuffle(q)*sinm
    cosT = np.ascontiguousarray(np.concatenate([cos.T, cos.T], 0))  # [128, S]
    sinm = sin.T.copy()                   # [DK, S]
    sinm[:32] = -sinm[:32]
    sinmT = np.ascontiguousarray(np.concatenate([sinm, sinm], 0))
    masks = _diag_masks()
    f = np.float32

    def bf(x):
        return np.ascontiguousarray(np.asarray(x, np.float32)).astype(BF_NP)

    def tile_w(w):
        # [D, M] -> [128(p), KC, M] per-partition contiguous
        d, m = w.shape
        return np.ascontiguousarray(
            np.asarray(w, f).reshape(KC, 128, m).transpose(1, 0, 2))

    wq_full = np.asarray(inputs["wq"], f)
    wk_full = np.asarray(inputs["wk"], f)
    wv_full = np.asarray(inputs["wv"], f)
    wo_full = np.asarray(inputs["wo_w"], f)
    ff1_full = np.asarray(inputs["ff1_w"], f)
    ff2_full = np.asarray(inputs["ff2_w"], f)
    outw_full = np.asarray(inputs["out_w"], f)
    outb_full = np.asarray(inputs["out_b"], f)

    # common (unsharded) tensors prepared once
    wo_t = bf(np.stack([tile_w(wo_full[l]) for l in range(L)]))
    # ff1: [L, mh, 128, KC, 128]
    ff1_t = np.stack([
        np.asarray(ff1_full[l], f).reshape(KC, 128, FF // 128, 128)
        .transpose(2, 1, 0, 3)
        for l in range(L)])
    ff1_t = bf(ff1_t)
    # ff1b: [L, 128, 32] with ff1b[l, p, m] = ff1_b[l, m*128+p]
    ff1b_t = np.ascontiguousarray(
        np.asarray(inputs["ff1_b"], f).reshape(L, FF // 128, 128)
        .transpose(0, 2, 1))
    # ff2: [L, g, 128, KC(within g), D]
    ff2_t = np.stack([
        np.asarray(ff2_full[l], f).reshape(4, KC, 128, D).transpose(0, 2, 1, 3)
        for l in range(L)])
    ff2_t = bf(ff2_t)
    temb_bf = bf(inputs["token_emb"])
    dmask_bf = masks.astype(BF_NP)
    cosT_bf = cosT.astype(BF_NP)
    sinmT_bf = sinmT.astype(BF_NP)

    in_maps = []
    for r in range(NC):
        hsl = slice(r * DHL, (r + 1) * DHL)
        # out_w shard -> [NVC, 128, KC, 512] padded
        ow = np.zeros((NVC, 128, KC, 512), np.float32)
        owr = outw_full[:, r * VL:(r + 1) * VL]          # [D, VL]
        owr_t = owr.reshape(KC, 128, VL).transpose(1, 0, 2)  # [128, KC, VL]
        ow[:, :, :, :] = 0.0
        for vc in range(NVC):
            vlen = VCL if vc == NVC - 1 else 512
            ow[vc, :, :, 0:vlen] = owr_t[:, :, vc * 512:vc * 512 + vlen]
        ob = np.zeros((NVC, 512), np.float32)
        obr = outb_full[r * VL:(r + 1) * VL]
        for vc in range(NVC):
            vlen = VCL if vc == NVC - 1 else 512
            ob[vc, 0:vlen] = obr[vc * 512:vc * 512 + vlen]

        im = {
            "ids": np.ascontiguousarray(
                ids[r * SL:(r + 1) * SL].reshape(2, 128, 1)),
            "token_emb": temb_bf,
            "pos_emb": np.ascontiguousarray(
                np.asarray(inputs["pos_emb"], f)[r * SL:(r + 1) * SL]),
            "wq": bf(np.stack([tile_w(wq_full[l][:, hsl])
                               for l in range(L)])),
            "wk": bf(np.stack([tile_w(wk_full[l][:, hsl])
                               for l in range(L)])),
            "wv": bf(np.stack([tile_w(wv_full[l][:, hsl])
                               for l in range(L)])),
            "wo_w": wo_t,
            "wo_b": np.asarray(inputs["wo_b"], f),
            "ln1_w": np.asarray(inputs["ln1_w"], f),
            "ln1_b": np.asarray(inputs["ln1_b"], f),
            "ln2_w": np.asarray(inputs["ln2_w"], f),
            "ln2_b": np.asarray(inputs["ln2_b"], f),
            "ff1_w": ff1_t,
            "ff1_b": ff1b_t,
            "ff2_w": ff2_t,
            "ff2_b": np.asarray(inputs["ff2_b"], f),
            "out_w": ow.astype(BF_NP),
            "out_b": ob,
            "cosT": cosT_bf,
            "sinmT": sinmT_bf,
            "dmask": dmask_bf,
        }
        in_maps.append(im)
    return in_maps


def run(inputs, n_layers=L, debug_x=False, **kw):
    key = (n_layers, debug_x)
    if key not in _CACHE:
        _CACHE[key] = build_program(n_layers, debug_x)
    nc = _CACHE[key]
    in_maps = _prepare_in_maps(inputs)
    res = run_bass_kernel_spmd(nc, in_maps, list(range(NC)), **kw)
    return res


def kernel(**inputs):
    res = run(inputs)
    logits = np.concatenate([res.results[r]["logits"] for r in range(NC)],
                            axis=1)
    return logits.reshape(B, S, V)
